# revision 7
# baseline (speedup 1.0000x reference)
"""Epipolar correlation layer on 8 Trainium2 NeuronCores.

Design (v2, patch-gather):
  Host computes sampling geometry exactly as the reference (fp32), then
  for each live pixel (any in-image sample among the 81 offsets) finds a
  fixed 18-row x 24-col texel window covering every live bilinear tap.
  Live pixels are compacted globally across all 4 batches and split
  evenly over the 8 cores (the gather source holds all 4 images, group-
  indexed within int16 range).

  Device per 128-pixel chunk: 18 dma_gather descriptors per pixel (one
  per window row, 4608 B each: 24 tight 96-ch bf16 texels, 768 B group
  stride), DVE broadcast-multiply by the pixel's imgL vector, and a
  segmented tensor_reduce over channels -> 432 window dot-products per
  pixel. Chunks are processed as 9-row halves with 4 gather buffers so
  descriptor issue overlaps DVE work. Gather completion uses the ring-
  capacity fence: a gather's descriptors are guaranteed retired once the
  second-next gather finishes issuing (SWDGE ring holds 128 descriptor
  groups < 2x73), so the consumer waits two gathers ahead; two dummy
  fence gathers cover the tail.

  Host then combines 4 bilinear taps per (offset, pixel) from the window
  dot-products with fp32 weights.
"""
import numpy as np
import ml_dtypes

import concourse.bass as bass
import concourse.bacc as bacc
import concourse.mybir as mybir
from concourse import bass_utils
from concourse.library_config import mlp

B, C, H, W = 4, 96, 96, 320
HW = H * W
MAXD = list(range(-4, 5))
MIND = list(range(-4, 5))
O = 81

ROWS = 18            # window rows per pixel
HR = 9               # rows per half-chunk
TEX = 24             # texels per window row
EL = TEX * 96        # 2304 bf16 elems per descriptor (4608 B)
NIH = HR * 128       # 1152 gather positions per half-chunk
NIH16 = NIH // 16    # 72
GSTRIDE = 7690       # groups per batch image (7680 + pad)
NGROUP = 4 * GSTRIDE + 16
POS = HR * TEX       # 216 dot outputs per pixel per half

f32 = mybir.dt.float32
bf16 = mybir.dt.bfloat16
i16 = mybir.dt.int16

_CACHE = {}


# ---------------------------------------------------------------- geometry
def _part1_jax(R, T, initial_flow):
    import jax
    import jax.numpy as jnp

    cpu = jax.devices("cpu")[0]

    def f(R, T, initial_flow):
        K = np.zeros((3, 3), np.float64)
        K[0, 0] = 0.89115971 * W
        K[0, 2] = 0.5 * W
        K[1, 1] = 1.18821287 * H
        K[1, 2] = 0.5 * H
        K[2, 2] = 1.0
        Kn = K.astype(np.float32)
        Ki = np.linalg.inv(K).astype(np.float32)
        jj, ii = np.meshgrid(np.arange(W), np.arange(H))
        pix_h = np.stack([jj, ii, np.ones_like(jj)], -1).reshape(-1, 3).astype(np.float32)
        pixel_dir = jnp.asarray(pix_h @ Ki.T)
        pixel_loc = jnp.asarray(np.stack([jj, ii], -1).astype(np.float32))
        Kj = jnp.asarray(Kn)
        KR = jnp.einsum('ij,bjk->bik', Kj, R)
        first_part = jnp.einsum('bij,nj->bni', KR, pixel_dir)
        second_part = jnp.einsum('ij,bjk->bik', Kj, T)[:, :, 0][:, None, :]

        def safe(d):
            return jnp.where(jnp.abs(d) < 1e-6, 1e-6, d)

        end_point = first_part[..., :2] / safe(first_part[..., 2:3])
        space_point = first_part * 10.0 + second_part
        project_point = space_point[..., :2] / safe(space_point[..., 2:3])
        diff = project_point - end_point
        para = diff / jnp.maximum(jnp.linalg.norm(diff, axis=-1, keepdims=True), 1e-12)
        perp = jnp.stack([-para[..., 1], para[..., 0]], axis=-1)
        para_r = para.reshape(B, H, W, 2)
        perp_r = perp.reshape(B, H, W, 2)
        end_r = end_point.reshape(B, H, W, 2)
        flow_point = pixel_loc[None] + jnp.transpose(initial_flow, (0, 2, 3, 1))
        nearest_k = jnp.sum((flow_point - end_r) * para_r, axis=3, keepdims=True)
        initial_loc = end_r + nearest_k * para_r
        epipolar_flow = jnp.transpose(initial_loc - pixel_loc[None], (0, 3, 1, 2))
        para_out = jnp.transpose(para_r, (0, 3, 1, 2))
        return initial_loc, para_r, perp_r, epipolar_flow, para_out

    with jax.default_device(cpu):
        args = [jax.device_put(np.asarray(x), cpu) for x in (R, T, initial_flow)]
        out = jax.jit(f, backend="cpu")(*args)
    return [np.asarray(x) for x in out]


def geometry(R, T, initial_flow):
    """Windows, gather groups, tap positions and weights (all fp32-exact
    vs the reference)."""
    initial_loc, para, perp, eflow, para_out = _part1_jax(R, T, initial_flow)
    initial_loc = initial_loc.reshape(B, HW, 2)
    para = para.reshape(B, HW, 2)
    perp = perp.reshape(B, HW, 2)
    offsets = np.array([[p, q] for p in MAXD for q in MIND], np.float32)  # (O,2)

    base = initial_loc + perp                                    # (B,HW,2)
    one, two, half = np.float32(1.0), np.float32(2.0), np.float32(0.5)
    Wf, Hf = np.float32(W), np.float32(H)
    # g = base + para_i*para + perp_i ; normalize/denormalize as reference
    pa_i = offsets[:, 0][None, :, None]                          # (1,O,1)
    pe_i = offsets[:, 1][None, :, None]
    gx = base[:, None, :, 0] + pa_i * para[:, None, :, 0] + pe_i  # (B,O,HW)
    gy = base[:, None, :, 1] + pa_i * para[:, None, :, 1] + pe_i
    gxn = two * gx / (Wf - one) - one
    gyn = two * gy / (Hf - one) - one
    gx = ((gxn + one) * Wf - one) * half
    gy = ((gyn + one) * Hf - one) * half
    gx = np.nan_to_num(gx, nan=1e9, posinf=1e9, neginf=-1e9)
    gy = np.nan_to_num(gy, nan=1e9, posinf=1e9, neginf=-1e9)
    x0 = np.floor(gx)
    y0 = np.floor(gy)
    wx = gx - x0
    wy = gy - y0

    in_x = (x0 >= 0) & (x0 <= W - 2)
    left = x0 == -1
    right = x0 == np.float32(W - 1)
    xlive = (x0 >= -1) & (x0 <= W - 1)
    ws0 = np.where(in_x, one - wx, np.where(left, wx, 0.0)).astype(np.float32)
    ws1 = np.where(in_x, wx, np.where(right, one - wx, 0.0)).astype(np.float32)
    xb = np.clip(x0, 0, W - 2).astype(np.int32)                  # (B,O,HW)

    ylive = [(y0 + r >= 0) & (y0 + r <= H - 1) for r in (0, 1)]
    yc = [np.clip(y0 + r, 0, H - 1).astype(np.int32) for r in (0, 1)]
    wrow = [np.where(ylive[r], (one - wy) if r == 0 else wy, 0.0).astype(np.float32)
            for r in (0, 1)]

    slive = [xlive & ylive[r] for r in (0, 1)]                   # (B,O,HW)
    anylive = slive[0] | slive[1]
    pixlive = anylive.any(axis=1)                                # (B,HW)

    big = np.int32(1 << 20)
    xb_m = np.where(anylive, xb, big)
    xmin = xb_m.min(axis=1)                                      # (B,HW)
    ys = [np.where(slive[r], yc[r], big) for r in (0, 1)]
    ymin = np.minimum(ys[0].min(axis=1), ys[1].min(axis=1))      # (B,HW)
    WX = np.clip((xmin // 4) * 4, 0, W - TEX).astype(np.int32)
    WY = np.clip(ymin, 0, H - ROWS).astype(np.int32)

    # taps: (B,O,2r,2s,HW) weight + position-in-window
    wt = np.empty((B, O, 2, 2, HW), np.float32)
    pos = np.zeros((B, O, 2, 2, HW), np.int16)
    for r in (0, 1):
        dy = yc[r] - WY[:, None]                                 # (B,O,HW)
        dx = xb - WX[:, None]
        p_base = dy * TEX + dx
        for s in (0, 1):
            w = wrow[r] * (ws0 if s == 0 else ws1)
            w = np.where(slive[r], w, 0.0)
            wt[:, :, r, s, :] = w
            pos[:, :, r, s, :] = np.where(w != 0, p_base + s, 0).astype(np.int16)
    wt /= np.float32(C)
    return eflow, para_out, pixlive, WX, WY, pos, wt


# ---------------------------------------------------------------- device
def build_program(nhalf):
    nc = bacc.Bacc("TRN2", debug=False)
    src_d = nc.dram_tensor("src", [NGROUP, 384], bf16, kind="ExternalInput")
    idx_d = nc.dram_tensor("idxs", [nhalf, 128, NIH16], i16, kind="ExternalInput")
    lv_d = nc.dram_tensor("lv", [nhalf, 128, 96], bf16, kind="ExternalInput")
    d_out = nc.dram_tensor("dvals", [nhalf, 128, POS], f32, kind="ExternalOutput")

    src_ap = bass.AP(src_d[:].tensor, 0, [[384, NGROUP - 12], [1, EL]])
    fence_ap = bass.AP(src_d[:].tensor, 0, [[384, NGROUP - 12], [1, 128]])

    G = [nc.alloc_sbuf_tensor(f"g{i}", [128, HR, EL], bf16) for i in range(4)]
    idx_s = [nc.alloc_sbuf_tensor(f"ix{i}", [128, NIH16], i16) for i in range(4)]
    lv_s = [nc.alloc_sbuf_tensor(f"lv{i}", [128, 96], bf16) for i in range(4)]
    d_s = [nc.alloc_sbuf_tensor(f"d{i}", [128, POS], f32) for i in range(4)]
    fence_s = nc.alloc_sbuf_tensor("fen", [128, HR, 128], bf16)

    s_idx = nc.alloc_semaphore("s_idx")
    s_lv = nc.alloc_semaphore("s_lv")
    s_g = nc.alloc_semaphore("s_g")
    s_v = nc.alloc_semaphore("s_v")
    s_out = nc.alloc_semaphore("s_out")
    s_mB = nc.alloc_semaphore("s_mB")
    PF = HR * EL

    with nc.Block() as blk:

        def g_aps(k, off, width):
            gt = G[k % 4][:].tensor
            return bass.AP(gt, off, [[PF, 128], [96, POS], [1, width]])

        def l_aps(k, off, width):
            lt = lv_s[k % 4][:].tensor
            return bass.AP(lt, off, [[96, 128], [0, POS], [1, width]])

        @blk.vector
        def _(v):
            for k in range(nhalf):
                v.wait_ge(s_g, 16 * (k + 3))   # gather k+2 issued => k retired
                v.wait_ge(s_lv, 16 * (k + 1))
                if k >= 4:
                    v.wait_ge(s_out, 16 * (k - 3))
                gA = g_aps(k, 0, 48)
                v.tensor_mul(gA, gA, l_aps(k, 0, 48))
                v.wait_ge(s_mB, k + 1)
                v.tensor_add(gA, gA, g_aps(k, 48, 48))   # pairs (c, c+48)
                gA2 = g_aps(k, 0, 24)
                v.tensor_add(gA2, gA2, g_aps(k, 24, 24))  # quads
                gA3 = g_aps(k, 0, 12)
                v.tensor_add(gA3, gA3, g_aps(k, 12, 12))  # octs
                v.tensor_reduce(
                    d_s[k % 4][:], gA3,
                    mybir.AxisListType.X, mybir.AluOpType.add,
                ).then_inc(s_v, 1)

        @blk.gpsimd
        def _(g):
            g.load_library(mlp)

            def mulB(j):
                gB = g_aps(j, 48, 48)
                g.wait_ge(s_lv, 16 * (j + 1))
                g.tensor_mul(gB, gB, l_aps(j, 48, 48)).then_inc(s_mB, 1)

            for k in range(nhalf):
                g.wait_ge(s_idx, 16 * (k + 1))
                if k >= 4:
                    g.wait_ge(s_v, k - 3)      # vector k-4 done with G[k%4]
                g.dma_gather(
                    G[k % 4][:], src_ap, idx_s[k % 4][:],
                    NIH, NIH, EL, elem_step=384, transpose=False,
                    single_packet=False,
                ).then_inc(s_g, 16)
                if k >= 2:
                    mulB(k - 2)
            for _ in range(2):                  # tail fences
                g.dma_gather(
                    fence_s[:], fence_ap, idx_s[(nhalf - 1) % 4][:],
                    NIH, NIH, 128, elem_step=384, transpose=False,
                    single_packet=False,
                ).then_inc(s_g, 16)
            mulB(nhalf - 2)
            mulB(nhalf - 1)

        @blk.sync
        def _(sy):
            for k in range(min(4, nhalf)):
                sy.dma_start(idx_s[k][:], idx_d[k]).then_inc(s_idx, 16)
                sy.dma_start(lv_s[k][:], lv_d[k]).then_inc(s_lv, 16)
            for k in range(nhalf):
                sy.wait_ge(s_v, k + 1)
                if k + 4 < nhalf:
                    sy.dma_start(idx_s[k % 4][:], idx_d[k + 4]).then_inc(s_idx, 16)
                    sy.dma_start(lv_s[k % 4][:], lv_d[k + 4]).then_inc(s_lv, 16)
                sy.dma_start(d_out[k], d_s[k % 4][:]).then_inc(s_out, 16)
            sy.wait_ge(s_out, 16 * nhalf)

    nc.compile()
    nc.finalize()
    return nc


# ---------------------------------------------------------------- host glue
def prepare(imgL, imgR, R, T, initial_flow):
    eflow, para_out, pixlive, WX, WY, pos, wt = geometry(R, T, initial_flow)

    # gather source: all 4 images, tight (HW, 96) bf16, 768B groups
    src = np.zeros((NGROUP, 384), ml_dtypes.bfloat16)
    for b in range(B):
        t = np.moveaxis(imgR[b], 0, -1).reshape(HW, 96).astype(ml_dtypes.bfloat16)
        src[b * GSTRIDE: b * GSTRIDE + HW // 4] = t.reshape(HW // 4, 384)

    # global live-pixel compaction
    blist, plist = np.nonzero(pixlive)                  # (nlive,) each
    nlive = blist.size
    ncH = max(1, -(-nlive // (8 * 128)))                # full chunks per core
    cap = 8 * ncH * 128
    bl = np.zeros(cap, np.int64)
    pl = np.zeros(cap, np.int64)
    valid = np.zeros(cap, bool)
    bl[:nlive] = blist
    pl[:nlive] = plist
    valid[:nlive] = True

    # entry -> 18 window-row group indices
    wx_e = WX[bl, pl].astype(np.int64)
    wy_e = WY[bl, pl].astype(np.int64)
    g0 = bl * GSTRIDE + (wy_e * W + wx_e) // 4          # (cap,)
    rows = g0[:, None] + np.arange(ROWS)[None, :] * (W // 4)   # (cap,18)
    rows = np.where(valid[:, None], rows, 0)
    assert rows.max() < 32767

    # imgL vectors per entry
    py = pl // W
    px = pl % W
    lv_all = imgL[bl, :, py, px].astype(ml_dtypes.bfloat16)     # (cap,96)

    nhalf = 2 * ncH
    in_maps = []
    percore = ncH * 128
    for c in range(8):
        sl = slice(c * percore, (c + 1) * percore)
        r_c = rows[sl].reshape(ncH, 128, 2, HR)          # (nc,px,half,9)
        lists = r_c.transpose(0, 2, 3, 1).reshape(nhalf, NIH).astype(np.int16)
        idx_w = lists.reshape(nhalf, NIH16, 16).transpose(0, 2, 1)
        idx_full = np.ascontiguousarray(np.tile(idx_w, (1, 8, 1)))
        lv_c = lv_all[sl].reshape(ncH, 128, 96)
        lv_h = np.ascontiguousarray(
            np.repeat(lv_c[:, None], 2, axis=1).reshape(nhalf, 128, 96))
        in_maps.append({"src": src, "idxs": idx_full, "lv": lv_h})

    ctx = dict(eflow=eflow, para_out=para_out, pos=pos, wt=wt,
               bl=bl, pl=pl, valid=valid, nlive=nlive, ncH=ncH,
               pixlive=pixlive)
    return in_maps, ctx


def combine(results, ctx):
    ncH = ctx["ncH"]
    cap = 8 * ncH * 128
    dfull = np.empty((cap, 2 * POS), np.float32)
    percore = ncH * 128
    for c in range(8):
        dv = results[c]["dvals"].reshape(ncH, 2, 128, POS)
        dfull[c * percore:(c + 1) * percore] = (
            dv.transpose(0, 2, 1, 3).reshape(percore, 2 * POS))

    # entry map (B,HW) -> compacted row (0 for dead; weight 0 there)
    entmap = np.zeros((B, HW), np.int64)
    bl, pl, valid = ctx["bl"], ctx["pl"], ctx["valid"]
    entmap[bl[valid], pl[valid]] = np.nonzero(valid)[0]

    pos = ctx["pos"]                                    # (B,O,2,2,HW) i16
    wt = ctx["wt"]                                      # (B,O,2,2,HW) f32
    pos_r = pos.transpose(0, 4, 1, 2, 3).reshape(B, HW, O * 4)
    wt_r = wt.transpose(0, 4, 1, 2, 3).reshape(B, HW, O * 4)
    vals = dfull[entmap[:, :, None], pos_r.astype(np.int64)]    # (B,HW,O*4)
    corr = (vals * wt_r).reshape(B, HW, O, 4).sum(axis=3)       # (B,HW,O)
    corr = corr.transpose(0, 2, 1)                              # (B,O,HW)

    out = np.empty((B, 4 + O, H, W), np.float32)
    out[:, 0:2] = ctx["eflow"]
    out[:, 2:4] = ctx["para_out"]
    out[:, 4:] = corr.reshape(B, O, H, W)
    return out


def kernel(imgL, imgR, R, T, initial_flow):
    imgL = np.asarray(imgL)
    imgR = np.asarray(imgR)
    R = np.asarray(R)
    T = np.asarray(T)
    initial_flow = np.asarray(initial_flow)

    in_maps, ctx = prepare(imgL, imgR, R, T, initial_flow)
    nhalf = 2 * ctx["ncH"]
    key = ("nc", nhalf)
    if key not in _CACHE:
        _CACHE[key] = build_program(nhalf)
    nc = _CACHE[key]
    _CACHE["last"] = (nc, in_maps)

    res = bass_utils.run_bass_kernel_spmd(nc, in_maps, core_ids=list(range(8)),
                                          trace=False)
    return combine(res.results, ctx)


# revision 9
# speedup vs baseline: 1.7312x; 1.7312x over previous
"""Epipolar correlation layer on 8 Trainium2 NeuronCores.

Design (v2, patch-gather):
  Host computes sampling geometry exactly as the reference (fp32), then
  for each live pixel (any in-image sample among the 81 offsets) finds a
  fixed 18-row x 24-col texel window covering every live bilinear tap.
  Live pixels are compacted globally across all 4 batches and split
  evenly over the 8 cores (the gather source holds all 4 images, group-
  indexed within int16 range).

  Device per 128-pixel chunk: 18 dma_gather descriptors per pixel (one
  per window row, 4608 B each: 24 tight 96-ch bf16 texels, 768 B group
  stride), DVE broadcast-multiply by the pixel's imgL vector, and a
  segmented tensor_reduce over channels -> 432 window dot-products per
  pixel. Chunks are processed as 9-row halves with 4 gather buffers so
  descriptor issue overlaps DVE work. Gather completion uses the ring-
  capacity fence: a gather's descriptors are guaranteed retired once the
  second-next gather finishes issuing (SWDGE ring holds 128 descriptor
  groups < 2x73), so the consumer waits two gathers ahead; two dummy
  fence gathers cover the tail.

  Host then combines 4 bilinear taps per (offset, pixel) from the window
  dot-products with fp32 weights.
"""
import numpy as np
import ml_dtypes

import concourse.bass as bass
import concourse.bacc as bacc
import concourse.mybir as mybir
from concourse import bass_utils
from concourse.library_config import mlp

B, C, H, W = 4, 96, 96, 320
HW = H * W
MAXD = list(range(-4, 5))
MIND = list(range(-4, 5))
O = 81

ROWS = 18            # window rows per pixel
HR = 9               # rows per half-chunk
TEX = 24             # texels per window row
EL = TEX * 96        # 2304 bf16 elems per descriptor (4608 B)
NIH = HR * 128       # 1152 gather positions per half-chunk
NIH16 = NIH // 16    # 72
GSTRIDE = 7690       # groups per batch image (7680 + pad)
NGROUP = 4 * GSTRIDE + 16
POS = HR * TEX       # 216 dot outputs per pixel per half

f32 = mybir.dt.float32
bf16 = mybir.dt.bfloat16
i16 = mybir.dt.int16

_CACHE = {}


# ---------------------------------------------------------------- geometry
def _part1_jax(R, T, initial_flow):
    import jax
    import jax.numpy as jnp

    cpu = jax.devices("cpu")[0]

    def f(R, T, initial_flow):
        K = np.zeros((3, 3), np.float64)
        K[0, 0] = 0.89115971 * W
        K[0, 2] = 0.5 * W
        K[1, 1] = 1.18821287 * H
        K[1, 2] = 0.5 * H
        K[2, 2] = 1.0
        Kn = K.astype(np.float32)
        Ki = np.linalg.inv(K).astype(np.float32)
        jj, ii = np.meshgrid(np.arange(W), np.arange(H))
        pix_h = np.stack([jj, ii, np.ones_like(jj)], -1).reshape(-1, 3).astype(np.float32)
        pixel_dir = jnp.asarray(pix_h @ Ki.T)
        pixel_loc = jnp.asarray(np.stack([jj, ii], -1).astype(np.float32))
        Kj = jnp.asarray(Kn)
        KR = jnp.einsum('ij,bjk->bik', Kj, R)
        first_part = jnp.einsum('bij,nj->bni', KR, pixel_dir)
        second_part = jnp.einsum('ij,bjk->bik', Kj, T)[:, :, 0][:, None, :]

        def safe(d):
            return jnp.where(jnp.abs(d) < 1e-6, 1e-6, d)

        end_point = first_part[..., :2] / safe(first_part[..., 2:3])
        space_point = first_part * 10.0 + second_part
        project_point = space_point[..., :2] / safe(space_point[..., 2:3])
        diff = project_point - end_point
        para = diff / jnp.maximum(jnp.linalg.norm(diff, axis=-1, keepdims=True), 1e-12)
        perp = jnp.stack([-para[..., 1], para[..., 0]], axis=-1)
        para_r = para.reshape(B, H, W, 2)
        perp_r = perp.reshape(B, H, W, 2)
        end_r = end_point.reshape(B, H, W, 2)
        flow_point = pixel_loc[None] + jnp.transpose(initial_flow, (0, 2, 3, 1))
        nearest_k = jnp.sum((flow_point - end_r) * para_r, axis=3, keepdims=True)
        initial_loc = end_r + nearest_k * para_r
        epipolar_flow = jnp.transpose(initial_loc - pixel_loc[None], (0, 3, 1, 2))
        para_out = jnp.transpose(para_r, (0, 3, 1, 2))
        return initial_loc, para_r, perp_r, epipolar_flow, para_out

    with jax.default_device(cpu):
        args = [jax.device_put(np.asarray(x), cpu) for x in (R, T, initial_flow)]
        out = jax.jit(f, backend="cpu")(*args)
    return [np.asarray(x) for x in out]


def geometry(R, T, initial_flow):
    """Windows, gather groups, tap positions and weights (all fp32-exact
    vs the reference)."""
    initial_loc, para, perp, eflow, para_out = _part1_jax(R, T, initial_flow)
    initial_loc = initial_loc.reshape(B, HW, 2)
    para = para.reshape(B, HW, 2)
    perp = perp.reshape(B, HW, 2)
    offsets = np.array([[p, q] for p in MAXD for q in MIND], np.float32)  # (O,2)

    base = initial_loc + perp                                    # (B,HW,2)
    one, two, half = np.float32(1.0), np.float32(2.0), np.float32(0.5)
    Wf, Hf = np.float32(W), np.float32(H)
    # g = base + para_i*para + perp_i ; normalize/denormalize as reference
    pa_i = offsets[:, 0][None, :, None]                          # (1,O,1)
    pe_i = offsets[:, 1][None, :, None]
    gx = base[:, None, :, 0] + pa_i * para[:, None, :, 0] + pe_i  # (B,O,HW)
    gy = base[:, None, :, 1] + pa_i * para[:, None, :, 1] + pe_i
    gxn = two * gx / (Wf - one) - one
    gyn = two * gy / (Hf - one) - one
    gx = ((gxn + one) * Wf - one) * half
    gy = ((gyn + one) * Hf - one) * half
    gx = np.nan_to_num(gx, nan=1e9, posinf=1e9, neginf=-1e9)
    gy = np.nan_to_num(gy, nan=1e9, posinf=1e9, neginf=-1e9)
    x0 = np.floor(gx)
    y0 = np.floor(gy)
    wx = gx - x0
    wy = gy - y0

    in_x = (x0 >= 0) & (x0 <= W - 2)
    left = x0 == -1
    right = x0 == np.float32(W - 1)
    xlive = (x0 >= -1) & (x0 <= W - 1)
    ws0 = np.where(in_x, one - wx, np.where(left, wx, 0.0)).astype(np.float32)
    ws1 = np.where(in_x, wx, np.where(right, one - wx, 0.0)).astype(np.float32)
    xb = np.clip(x0, 0, W - 2).astype(np.int32)                  # (B,O,HW)

    ylive = [(y0 + r >= 0) & (y0 + r <= H - 1) for r in (0, 1)]
    yc = [np.clip(y0 + r, 0, H - 1).astype(np.int32) for r in (0, 1)]
    wrow = [np.where(ylive[r], (one - wy) if r == 0 else wy, 0.0).astype(np.float32)
            for r in (0, 1)]

    slive = [xlive & ylive[r] for r in (0, 1)]                   # (B,O,HW)
    anylive = slive[0] | slive[1]
    pixlive = anylive.any(axis=1)                                # (B,HW)

    big = np.int32(1 << 20)
    xb_m = np.where(anylive, xb, big)
    xmin = xb_m.min(axis=1)                                      # (B,HW)
    ys = [np.where(slive[r], yc[r], big) for r in (0, 1)]
    ymin = np.minimum(ys[0].min(axis=1), ys[1].min(axis=1))      # (B,HW)
    WX = np.clip((xmin // 4) * 4, 0, W - TEX).astype(np.int32)
    WY = np.clip(ymin, 0, H - ROWS).astype(np.int32)

    # taps: (B,O,2r,2s,HW) weight + position-in-window
    wt = np.empty((B, O, 2, 2, HW), np.float32)
    pos = np.zeros((B, O, 2, 2, HW), np.int16)
    for r in (0, 1):
        dy = yc[r] - WY[:, None]                                 # (B,O,HW)
        dx = xb - WX[:, None]
        p_base = dy * TEX + dx
        for s in (0, 1):
            w = wrow[r] * (ws0 if s == 0 else ws1)
            w = np.where(slive[r], w, 0.0)
            wt[:, :, r, s, :] = w
            pos[:, :, r, s, :] = np.where(w != 0, p_base + s, 0).astype(np.int16)
    wt /= np.float32(C)
    return eflow, para_out, pixlive, WX, WY, pos, wt


# ---------------------------------------------------------------- device
def build_program(nhalf):
    nc = bacc.Bacc("TRN2", debug=False)
    src_d = nc.dram_tensor("src", [NGROUP, 384], bf16, kind="ExternalInput")
    idx_d = nc.dram_tensor("idxs", [nhalf, 128, NIH16], i16, kind="ExternalInput")
    lv_d = nc.dram_tensor("lv", [nhalf, 128, 96], bf16, kind="ExternalInput")
    d_out = nc.dram_tensor("dvals", [nhalf, 128, POS], f32, kind="ExternalOutput")

    src_ap = bass.AP(src_d[:].tensor, 0, [[384, NGROUP - 12], [1, EL]])
    fence_ap = bass.AP(src_d[:].tensor, 0, [[384, NGROUP - 12], [1, 128]])

    G = [nc.alloc_sbuf_tensor(f"g{i}", [128, HR, EL], bf16) for i in range(4)]
    idx_s = [nc.alloc_sbuf_tensor(f"ix{i}", [128, NIH16], i16) for i in range(4)]
    lv_s = [nc.alloc_sbuf_tensor(f"lv{i}", [128, 96], bf16) for i in range(4)]
    d_s = [nc.alloc_sbuf_tensor(f"d{i}", [128, POS], f32) for i in range(4)]
    fence_s = nc.alloc_sbuf_tensor("fen", [128, HR, 128], bf16)

    s_idx = nc.alloc_semaphore("s_idx")
    s_lv = nc.alloc_semaphore("s_lv")
    s_g = nc.alloc_semaphore("s_g")
    s_v = nc.alloc_semaphore("s_v")
    s_out = nc.alloc_semaphore("s_out")
    s_mB = nc.alloc_semaphore("s_mB")
    PF = HR * EL

    with nc.Block() as blk:

        def g_aps(k, off, width):
            gt = G[k % 4][:].tensor
            return bass.AP(gt, off, [[PF, 128], [96, POS], [1, width]])

        def l_aps(k, off, width):
            lt = lv_s[k % 4][:].tensor
            return bass.AP(lt, off, [[96, 128], [0, POS], [1, width]])

        @blk.vector
        def _(v):
            for k in range(nhalf):
                v.wait_ge(s_g, 16 * (k + 3))   # gather k+2 issued => k retired
                v.wait_ge(s_lv, 16 * (k + 1))
                if k >= 4:
                    v.wait_ge(s_out, 16 * (k - 3))
                gA = g_aps(k, 0, 48)
                v.tensor_mul(gA, gA, l_aps(k, 0, 48))
                gB = g_aps(k, 48, 48)
                v.tensor_mul(gB, gB, l_aps(k, 48, 48))
                v.tensor_add(gA, gA, gB)                 # pairs (c, c+48)
                gA2 = g_aps(k, 0, 24)
                v.tensor_add(gA2, gA2, g_aps(k, 24, 24))  # quads
                gA3 = g_aps(k, 0, 12)
                v.tensor_add(gA3, gA3, g_aps(k, 12, 12))  # octs
                v.tensor_reduce(
                    d_s[k % 4][:], gA3,
                    mybir.AxisListType.X, mybir.AluOpType.add,
                ).then_inc(s_v, 1)

        @blk.gpsimd
        def _(g):
            g.load_library(mlp)
            for k in range(nhalf):
                g.wait_ge(s_idx, 16 * (k + 1))
                if k >= 4:
                    g.wait_ge(s_v, k - 3)      # vector k-4 done with G[k%4]
                g.dma_gather(
                    G[k % 4][:], src_ap, idx_s[k % 4][:],
                    NIH, NIH, EL, elem_step=384, transpose=False,
                    single_packet=False,
                ).then_inc(s_g, 16)
            for _ in range(2):                  # tail fences
                g.dma_gather(
                    fence_s[:], fence_ap, idx_s[(nhalf - 1) % 4][:],
                    NIH, NIH, 128, elem_step=384, transpose=False,
                    single_packet=False,
                ).then_inc(s_g, 16)

        @blk.sync
        def _(sy):
            for k in range(min(4, nhalf)):
                sy.dma_start(idx_s[k][:], idx_d[k]).then_inc(s_idx, 16)
                sy.dma_start(lv_s[k][:], lv_d[k]).then_inc(s_lv, 16)
            for k in range(nhalf):
                sy.wait_ge(s_v, k + 1)
                if k + 4 < nhalf:
                    sy.dma_start(idx_s[k % 4][:], idx_d[k + 4]).then_inc(s_idx, 16)
                    sy.dma_start(lv_s[k % 4][:], lv_d[k + 4]).then_inc(s_lv, 16)
                sy.dma_start(d_out[k], d_s[k % 4][:]).then_inc(s_out, 16)
            sy.wait_ge(s_out, 16 * nhalf)

    nc.compile()
    nc.finalize()
    return nc


# ---------------------------------------------------------------- host glue
def prepare(imgL, imgR, R, T, initial_flow):
    eflow, para_out, pixlive, WX, WY, pos, wt = geometry(R, T, initial_flow)

    # gather source: all 4 images, tight (HW, 96) bf16, 768B groups
    src = np.zeros((NGROUP, 384), ml_dtypes.bfloat16)
    for b in range(B):
        t = np.moveaxis(imgR[b], 0, -1).reshape(HW, 96).astype(ml_dtypes.bfloat16)
        src[b * GSTRIDE: b * GSTRIDE + HW // 4] = t.reshape(HW // 4, 384)

    # global live-pixel compaction
    blist, plist = np.nonzero(pixlive)                  # (nlive,) each
    nlive = blist.size
    ncH = max(1, -(-nlive // (8 * 128)))                # full chunks per core
    cap = 8 * ncH * 128
    bl = np.zeros(cap, np.int64)
    pl = np.zeros(cap, np.int64)
    valid = np.zeros(cap, bool)
    bl[:nlive] = blist
    pl[:nlive] = plist
    valid[:nlive] = True

    # entry -> 18 window-row group indices
    wx_e = WX[bl, pl].astype(np.int64)
    wy_e = WY[bl, pl].astype(np.int64)
    g0 = bl * GSTRIDE + (wy_e * W + wx_e) // 4          # (cap,)
    rows = g0[:, None] + np.arange(ROWS)[None, :] * (W // 4)   # (cap,18)
    rows = np.where(valid[:, None], rows, 0)
    assert rows.max() < 32767

    # imgL vectors per entry
    py = pl // W
    px = pl % W
    lv_all = imgL[bl, :, py, px].astype(ml_dtypes.bfloat16)     # (cap,96)

    nhalf = 2 * ncH
    in_maps = []
    percore = ncH * 128
    for c in range(8):
        sl = slice(c * percore, (c + 1) * percore)
        r_c = rows[sl].reshape(ncH, 128, 2, HR)          # (nc,px,half,9)
        lists = r_c.transpose(0, 2, 3, 1).reshape(nhalf, NIH).astype(np.int16)
        idx_w = lists.reshape(nhalf, NIH16, 16).transpose(0, 2, 1)
        idx_full = np.ascontiguousarray(np.tile(idx_w, (1, 8, 1)))
        lv_c = lv_all[sl].reshape(ncH, 128, 96)
        lv_h = np.ascontiguousarray(
            np.repeat(lv_c[:, None], 2, axis=1).reshape(nhalf, 128, 96))
        in_maps.append({"src": src, "idxs": idx_full, "lv": lv_h})

    ctx = dict(eflow=eflow, para_out=para_out, pos=pos, wt=wt,
               bl=bl, pl=pl, valid=valid, nlive=nlive, ncH=ncH,
               pixlive=pixlive)
    return in_maps, ctx


def combine(results, ctx):
    ncH = ctx["ncH"]
    cap = 8 * ncH * 128
    dfull = np.empty((cap, 2 * POS), np.float32)
    percore = ncH * 128
    for c in range(8):
        dv = results[c]["dvals"].reshape(ncH, 2, 128, POS)
        dfull[c * percore:(c + 1) * percore] = (
            dv.transpose(0, 2, 1, 3).reshape(percore, 2 * POS))

    # entry map (B,HW) -> compacted row (0 for dead; weight 0 there)
    entmap = np.zeros((B, HW), np.int64)
    bl, pl, valid = ctx["bl"], ctx["pl"], ctx["valid"]
    entmap[bl[valid], pl[valid]] = np.nonzero(valid)[0]

    pos = ctx["pos"]                                    # (B,O,2,2,HW) i16
    wt = ctx["wt"]                                      # (B,O,2,2,HW) f32
    pos_r = pos.transpose(0, 4, 1, 2, 3).reshape(B, HW, O * 4)
    wt_r = wt.transpose(0, 4, 1, 2, 3).reshape(B, HW, O * 4)
    vals = dfull[entmap[:, :, None], pos_r.astype(np.int64)]    # (B,HW,O*4)
    corr = (vals * wt_r).reshape(B, HW, O, 4).sum(axis=3)       # (B,HW,O)
    corr = corr.transpose(0, 2, 1)                              # (B,O,HW)

    out = np.empty((B, 4 + O, H, W), np.float32)
    out[:, 0:2] = ctx["eflow"]
    out[:, 2:4] = ctx["para_out"]
    out[:, 4:] = corr.reshape(B, O, H, W)
    return out


def kernel(imgL, imgR, R, T, initial_flow):
    imgL = np.asarray(imgL)
    imgR = np.asarray(imgR)
    R = np.asarray(R)
    T = np.asarray(T)
    initial_flow = np.asarray(initial_flow)

    in_maps, ctx = prepare(imgL, imgR, R, T, initial_flow)
    nhalf = 2 * ctx["ncH"]
    key = ("nc", nhalf)
    if key not in _CACHE:
        _CACHE[key] = build_program(nhalf)
    nc = _CACHE[key]
    _CACHE["last"] = (nc, in_maps)

    res = bass_utils.run_bass_kernel_spmd(nc, in_maps, core_ids=list(range(8)),
                                          trace=False)
    return combine(res.results, ctx)


# revision 10
# speedup vs baseline: 1.9138x; 1.1054x over previous
"""Epipolar correlation layer on 8 Trainium2 NeuronCores.

Design (v2, patch-gather):
  Host computes sampling geometry exactly as the reference (fp32), then
  for each live pixel (any in-image sample among the 81 offsets) finds a
  fixed 18-row x 24-col texel window covering every live bilinear tap.
  Live pixels are compacted globally across all 4 batches and split
  evenly over the 8 cores (the gather source holds all 4 images, group-
  indexed within int16 range).

  Device per 128-pixel chunk: 18 dma_gather descriptors per pixel (one
  per window row, 4608 B each: 24 tight 96-ch bf16 texels, 768 B group
  stride), DVE broadcast-multiply by the pixel's imgL vector, and a
  segmented tensor_reduce over channels -> 432 window dot-products per
  pixel. Chunks are processed as 9-row halves with 4 gather buffers so
  descriptor issue overlaps DVE work. Gather completion uses the ring-
  capacity fence: a gather's descriptors are guaranteed retired once the
  second-next gather finishes issuing (SWDGE ring holds 128 descriptor
  groups < 2x73), so the consumer waits two gathers ahead; two dummy
  fence gathers cover the tail.

  Host then combines 4 bilinear taps per (offset, pixel) from the window
  dot-products with fp32 weights.
"""
import numpy as np
import ml_dtypes

import concourse.bass as bass
import concourse.bacc as bacc
import concourse.mybir as mybir
from concourse import bass_utils
from concourse.library_config import mlp

B, C, H, W = 4, 96, 96, 320
HW = H * W
MAXD = list(range(-4, 5))
MIND = list(range(-4, 5))
O = 81

ROWS = 18            # window rows per pixel
HR = 9               # rows per half-chunk
TEX = 24             # texels per window row
EL = TEX * 96        # 2304 bf16 elems per descriptor (4608 B)
NIH = HR * 128       # 1152 gather positions per half-chunk
NIH16 = NIH // 16    # 72
GSTRIDE = 7690       # groups per batch image (7680 + pad)
NGROUP = 4 * GSTRIDE + 16
POS = HR * TEX       # 216 dot outputs per pixel per half

f32 = mybir.dt.float32
bf16 = mybir.dt.bfloat16
i16 = mybir.dt.int16

_CACHE = {}


# ---------------------------------------------------------------- geometry
def _part1_jax(R, T, initial_flow):
    import jax
    import jax.numpy as jnp

    cpu = jax.devices("cpu")[0]

    def f(R, T, initial_flow):
        K = np.zeros((3, 3), np.float64)
        K[0, 0] = 0.89115971 * W
        K[0, 2] = 0.5 * W
        K[1, 1] = 1.18821287 * H
        K[1, 2] = 0.5 * H
        K[2, 2] = 1.0
        Kn = K.astype(np.float32)
        Ki = np.linalg.inv(K).astype(np.float32)
        jj, ii = np.meshgrid(np.arange(W), np.arange(H))
        pix_h = np.stack([jj, ii, np.ones_like(jj)], -1).reshape(-1, 3).astype(np.float32)
        pixel_dir = jnp.asarray(pix_h @ Ki.T)
        pixel_loc = jnp.asarray(np.stack([jj, ii], -1).astype(np.float32))
        Kj = jnp.asarray(Kn)
        KR = jnp.einsum('ij,bjk->bik', Kj, R)
        first_part = jnp.einsum('bij,nj->bni', KR, pixel_dir)
        second_part = jnp.einsum('ij,bjk->bik', Kj, T)[:, :, 0][:, None, :]

        def safe(d):
            return jnp.where(jnp.abs(d) < 1e-6, 1e-6, d)

        end_point = first_part[..., :2] / safe(first_part[..., 2:3])
        space_point = first_part * 10.0 + second_part
        project_point = space_point[..., :2] / safe(space_point[..., 2:3])
        diff = project_point - end_point
        para = diff / jnp.maximum(jnp.linalg.norm(diff, axis=-1, keepdims=True), 1e-12)
        perp = jnp.stack([-para[..., 1], para[..., 0]], axis=-1)
        para_r = para.reshape(B, H, W, 2)
        perp_r = perp.reshape(B, H, W, 2)
        end_r = end_point.reshape(B, H, W, 2)
        flow_point = pixel_loc[None] + jnp.transpose(initial_flow, (0, 2, 3, 1))
        nearest_k = jnp.sum((flow_point - end_r) * para_r, axis=3, keepdims=True)
        initial_loc = end_r + nearest_k * para_r
        epipolar_flow = jnp.transpose(initial_loc - pixel_loc[None], (0, 3, 1, 2))
        para_out = jnp.transpose(para_r, (0, 3, 1, 2))
        return initial_loc, para_r, perp_r, epipolar_flow, para_out

    with jax.default_device(cpu):
        args = [jax.device_put(np.asarray(x), cpu) for x in (R, T, initial_flow)]
        out = jax.jit(f, backend="cpu")(*args)
    return [np.asarray(x) for x in out]


def geometry(R, T, initial_flow):
    """Windows, gather groups, tap positions and weights (all fp32-exact
    vs the reference)."""
    initial_loc, para, perp, eflow, para_out = _part1_jax(R, T, initial_flow)
    initial_loc = initial_loc.reshape(B, HW, 2)
    para = para.reshape(B, HW, 2)
    perp = perp.reshape(B, HW, 2)
    offsets = np.array([[p, q] for p in MAXD for q in MIND], np.float32)  # (O,2)

    base = initial_loc + perp                                    # (B,HW,2)
    one, two, half = np.float32(1.0), np.float32(2.0), np.float32(0.5)
    Wf, Hf = np.float32(W), np.float32(H)
    # g = base + para_i*para + perp_i ; normalize/denormalize as reference
    pa_i = offsets[:, 0][None, :, None]                          # (1,O,1)
    pe_i = offsets[:, 1][None, :, None]
    gx = base[:, None, :, 0] + pa_i * para[:, None, :, 0] + pe_i  # (B,O,HW)
    gy = base[:, None, :, 1] + pa_i * para[:, None, :, 1] + pe_i
    gxn = two * gx / (Wf - one) - one
    gyn = two * gy / (Hf - one) - one
    gx = ((gxn + one) * Wf - one) * half
    gy = ((gyn + one) * Hf - one) * half
    gx = np.nan_to_num(gx, nan=1e9, posinf=1e9, neginf=-1e9)
    gy = np.nan_to_num(gy, nan=1e9, posinf=1e9, neginf=-1e9)
    x0 = np.floor(gx)
    y0 = np.floor(gy)
    wx = gx - x0
    wy = gy - y0

    in_x = (x0 >= 0) & (x0 <= W - 2)
    left = x0 == -1
    right = x0 == np.float32(W - 1)
    xlive = (x0 >= -1) & (x0 <= W - 1)
    ws0 = np.where(in_x, one - wx, np.where(left, wx, 0.0)).astype(np.float32)
    ws1 = np.where(in_x, wx, np.where(right, one - wx, 0.0)).astype(np.float32)
    xb = np.clip(x0, 0, W - 2).astype(np.int32)                  # (B,O,HW)

    ylive = [(y0 + r >= 0) & (y0 + r <= H - 1) for r in (0, 1)]
    yc = [np.clip(y0 + r, 0, H - 1).astype(np.int32) for r in (0, 1)]
    wrow = [np.where(ylive[r], (one - wy) if r == 0 else wy, 0.0).astype(np.float32)
            for r in (0, 1)]

    slive = [xlive & ylive[r] for r in (0, 1)]                   # (B,O,HW)
    anylive = slive[0] | slive[1]
    pixlive = anylive.any(axis=1)                                # (B,HW)

    big = np.int32(1 << 20)
    xb_m = np.where(anylive, xb, big)
    xmin = xb_m.min(axis=1)                                      # (B,HW)
    ys = [np.where(slive[r], yc[r], big) for r in (0, 1)]
    ymin = np.minimum(ys[0].min(axis=1), ys[1].min(axis=1))      # (B,HW)
    WX = np.clip((xmin // 4) * 4, 0, W - TEX).astype(np.int32)
    WY = np.clip(ymin, 0, H - ROWS).astype(np.int32)

    # taps: (B,O,2r,2s,HW) weight + position-in-window
    wt = np.empty((B, O, 2, 2, HW), np.float32)
    pos = np.zeros((B, O, 2, 2, HW), np.int16)
    for r in (0, 1):
        dy = yc[r] - WY[:, None]                                 # (B,O,HW)
        dx = xb - WX[:, None]
        p_base = dy * TEX + dx
        for s in (0, 1):
            w = wrow[r] * (ws0 if s == 0 else ws1)
            w = np.where(slive[r], w, 0.0)
            wt[:, :, r, s, :] = w
            pos[:, :, r, s, :] = np.where(w != 0, p_base + s, 0).astype(np.int16)
    wt /= np.float32(C)
    return eflow, para_out, pixlive, WX, WY, pos, wt


# ---------------------------------------------------------------- device
def build_program(nhalf):
    nc = bacc.Bacc("TRN2", debug=False)
    src_d = nc.dram_tensor("src", [NGROUP, 384], bf16, kind="ExternalInput")
    idx_d = nc.dram_tensor("idxs", [nhalf, 128, NIH16], i16, kind="ExternalInput")
    lv_d = nc.dram_tensor("lv", [nhalf, 128, 96], bf16, kind="ExternalInput")
    d_out = nc.dram_tensor("dvals", [nhalf, 128, POS], f32, kind="ExternalOutput")

    src_ap = bass.AP(src_d[:].tensor, 0, [[384, NGROUP - 12], [1, EL]])
    fence_ap = bass.AP(src_d[:].tensor, 0, [[384, NGROUP - 12], [1, 128]])

    G = [nc.alloc_sbuf_tensor(f"g{i}", [128, HR, EL], bf16) for i in range(4)]
    idx_s = [nc.alloc_sbuf_tensor(f"ix{i}", [128, NIH16], i16) for i in range(4)]
    lv_s = [nc.alloc_sbuf_tensor(f"lv{i}", [128, 96], bf16) for i in range(4)]
    d_s = [nc.alloc_sbuf_tensor(f"d{i}", [128, POS], f32) for i in range(4)]
    fence_s = nc.alloc_sbuf_tensor("fen", [128, HR, 128], bf16)

    s_idx = nc.alloc_semaphore("s_idx")
    s_lv = nc.alloc_semaphore("s_lv")
    s_g = nc.alloc_semaphore("s_g")
    s_v = nc.alloc_semaphore("s_v")
    s_out = nc.alloc_semaphore("s_out")
    s_mB = nc.alloc_semaphore("s_mB")
    PF = HR * EL

    with nc.Block() as blk:

        def g_aps(k, off, width):
            gt = G[k % 4][:].tensor
            return bass.AP(gt, off, [[PF, 128], [96, POS], [1, width]])

        def l_aps(k, off, width):
            lt = lv_s[k % 4][:].tensor
            return bass.AP(lt, off, [[96, 128], [0, POS], [1, width]])

        @blk.vector
        def _(v):
            for k in range(nhalf):
                v.wait_ge(s_g, 16 * (k + 3))   # gather k+2 issued => k retired
                v.wait_ge(s_lv, 16 * (k + 1))
                if k >= 4:
                    v.wait_ge(s_out, 16 * (k - 3))
                gA = g_aps(k, 0, 48)
                v.tensor_mul(gA, gA, l_aps(k, 0, 48))
                gB = g_aps(k, 48, 48)
                v.tensor_mul(gB, gB, l_aps(k, 48, 48))
                v.tensor_add(gA, gA, gB)                 # pairs (c, c+48)
                gA2 = g_aps(k, 0, 24)
                v.tensor_add(gA2, gA2, g_aps(k, 24, 24))  # quads
                v.tensor_reduce(
                    d_s[k % 4][:], gA2,
                    mybir.AxisListType.X, mybir.AluOpType.add,
                ).then_inc(s_v, 1)

        @blk.gpsimd
        def _(g):
            g.load_library(mlp)
            for k in range(nhalf):
                g.wait_ge(s_idx, 16 * (k + 1))
                if k >= 4:
                    g.wait_ge(s_v, k - 3)      # vector k-4 done with G[k%4]
                g.dma_gather(
                    G[k % 4][:], src_ap, idx_s[k % 4][:],
                    NIH, NIH, EL, elem_step=384, transpose=False,
                    single_packet=False,
                ).then_inc(s_g, 16)
            for _ in range(2):                  # tail fences
                g.dma_gather(
                    fence_s[:], fence_ap, idx_s[(nhalf - 1) % 4][:],
                    NIH, NIH, 128, elem_step=384, transpose=False,
                    single_packet=False,
                ).then_inc(s_g, 16)

        @blk.sync
        def _(sy):
            for k in range(min(4, nhalf)):
                sy.dma_start(idx_s[k][:], idx_d[k]).then_inc(s_idx, 16)
                sy.dma_start(lv_s[k][:], lv_d[k]).then_inc(s_lv, 16)
            for k in range(nhalf):
                sy.wait_ge(s_v, k + 1)
                if k + 4 < nhalf:
                    sy.dma_start(idx_s[k % 4][:], idx_d[k + 4]).then_inc(s_idx, 16)
                    sy.dma_start(lv_s[k % 4][:], lv_d[k + 4]).then_inc(s_lv, 16)
                sy.dma_start(d_out[k], d_s[k % 4][:]).then_inc(s_out, 16)
            sy.wait_ge(s_out, 16 * nhalf)

    nc.compile()
    nc.finalize()
    return nc


# ---------------------------------------------------------------- host glue
def prepare(imgL, imgR, R, T, initial_flow):
    eflow, para_out, pixlive, WX, WY, pos, wt = geometry(R, T, initial_flow)

    # gather source: all 4 images, tight (HW, 96) bf16, 768B groups
    src = np.zeros((NGROUP, 384), ml_dtypes.bfloat16)
    for b in range(B):
        t = np.moveaxis(imgR[b], 0, -1).reshape(HW, 96).astype(ml_dtypes.bfloat16)
        src[b * GSTRIDE: b * GSTRIDE + HW // 4] = t.reshape(HW // 4, 384)

    # global live-pixel compaction
    blist, plist = np.nonzero(pixlive)                  # (nlive,) each
    nlive = blist.size
    ncH = max(1, -(-nlive // (8 * 128)))                # full chunks per core
    cap = 8 * ncH * 128
    bl = np.zeros(cap, np.int64)
    pl = np.zeros(cap, np.int64)
    valid = np.zeros(cap, bool)
    bl[:nlive] = blist
    pl[:nlive] = plist
    valid[:nlive] = True

    # entry -> 18 window-row group indices
    wx_e = WX[bl, pl].astype(np.int64)
    wy_e = WY[bl, pl].astype(np.int64)
    g0 = bl * GSTRIDE + (wy_e * W + wx_e) // 4          # (cap,)
    rows = g0[:, None] + np.arange(ROWS)[None, :] * (W // 4)   # (cap,18)
    rows = np.where(valid[:, None], rows, 0)
    assert rows.max() < 32767

    # imgL vectors per entry
    py = pl // W
    px = pl % W
    lv_all = imgL[bl, :, py, px].astype(ml_dtypes.bfloat16)     # (cap,96)

    nhalf = 2 * ncH
    in_maps = []
    percore = ncH * 128
    for c in range(8):
        sl = slice(c * percore, (c + 1) * percore)
        r_c = rows[sl].reshape(ncH, 128, 2, HR)          # (nc,px,half,9)
        lists = r_c.transpose(0, 2, 3, 1).reshape(nhalf, NIH).astype(np.int16)
        idx_w = lists.reshape(nhalf, NIH16, 16).transpose(0, 2, 1)
        idx_full = np.ascontiguousarray(np.tile(idx_w, (1, 8, 1)))
        lv_c = lv_all[sl].reshape(ncH, 128, 96)
        lv_h = np.ascontiguousarray(
            np.repeat(lv_c[:, None], 2, axis=1).reshape(nhalf, 128, 96))
        in_maps.append({"src": src, "idxs": idx_full, "lv": lv_h})

    ctx = dict(eflow=eflow, para_out=para_out, pos=pos, wt=wt,
               bl=bl, pl=pl, valid=valid, nlive=nlive, ncH=ncH,
               pixlive=pixlive)
    return in_maps, ctx


def combine(results, ctx):
    ncH = ctx["ncH"]
    cap = 8 * ncH * 128
    dfull = np.empty((cap, 2 * POS), np.float32)
    percore = ncH * 128
    for c in range(8):
        dv = results[c]["dvals"].reshape(ncH, 2, 128, POS)
        dfull[c * percore:(c + 1) * percore] = (
            dv.transpose(0, 2, 1, 3).reshape(percore, 2 * POS))

    # entry map (B,HW) -> compacted row (0 for dead; weight 0 there)
    entmap = np.zeros((B, HW), np.int64)
    bl, pl, valid = ctx["bl"], ctx["pl"], ctx["valid"]
    entmap[bl[valid], pl[valid]] = np.nonzero(valid)[0]

    pos = ctx["pos"]                                    # (B,O,2,2,HW) i16
    wt = ctx["wt"]                                      # (B,O,2,2,HW) f32
    pos_r = pos.transpose(0, 4, 1, 2, 3).reshape(B, HW, O * 4)
    wt_r = wt.transpose(0, 4, 1, 2, 3).reshape(B, HW, O * 4)
    vals = dfull[entmap[:, :, None], pos_r.astype(np.int64)]    # (B,HW,O*4)
    corr = (vals * wt_r).reshape(B, HW, O, 4).sum(axis=3)       # (B,HW,O)
    corr = corr.transpose(0, 2, 1)                              # (B,O,HW)

    out = np.empty((B, 4 + O, H, W), np.float32)
    out[:, 0:2] = ctx["eflow"]
    out[:, 2:4] = ctx["para_out"]
    out[:, 4:] = corr.reshape(B, O, H, W)
    return out


def kernel(imgL, imgR, R, T, initial_flow):
    imgL = np.asarray(imgL)
    imgR = np.asarray(imgR)
    R = np.asarray(R)
    T = np.asarray(T)
    initial_flow = np.asarray(initial_flow)

    in_maps, ctx = prepare(imgL, imgR, R, T, initial_flow)
    nhalf = 2 * ctx["ncH"]
    key = ("nc", nhalf)
    if key not in _CACHE:
        _CACHE[key] = build_program(nhalf)
    nc = _CACHE[key]
    _CACHE["last"] = (nc, in_maps)

    res = bass_utils.run_bass_kernel_spmd(nc, in_maps, core_ids=list(range(8)),
                                          trace=False)
    return combine(res.results, ctx)


# revision 11
# speedup vs baseline: 1.9482x; 1.0180x over previous
"""Epipolar correlation layer on 8 Trainium2 NeuronCores.

Design (v2, patch-gather):
  Host computes sampling geometry exactly as the reference (fp32), then
  for each live pixel (any in-image sample among the 81 offsets) finds a
  fixed 18-row x 24-col texel window covering every live bilinear tap.
  Live pixels are compacted globally across all 4 batches and split
  evenly over the 8 cores (the gather source holds all 4 images, group-
  indexed within int16 range).

  Device per 128-pixel chunk: 18 dma_gather descriptors per pixel (one
  per window row, 4608 B each: 24 tight 96-ch bf16 texels, 768 B group
  stride), DVE broadcast-multiply by the pixel's imgL vector, and a
  segmented tensor_reduce over channels -> 432 window dot-products per
  pixel. Chunks are processed as 9-row halves with 4 gather buffers so
  descriptor issue overlaps DVE work. Gather completion uses the ring-
  capacity fence: a gather's descriptors are guaranteed retired once the
  second-next gather finishes issuing (SWDGE ring holds 128 descriptor
  groups < 2x73), so the consumer waits two gathers ahead; two dummy
  fence gathers cover the tail.

  Host then combines 4 bilinear taps per (offset, pixel) from the window
  dot-products with fp32 weights.
"""
import numpy as np
import ml_dtypes

import concourse.bass as bass
import concourse.bacc as bacc
import concourse.mybir as mybir
from concourse import bass_utils
from concourse.library_config import mlp

B, C, H, W = 4, 96, 96, 320
HW = H * W
MAXD = list(range(-4, 5))
MIND = list(range(-4, 5))
O = 81

ROWS = 18            # window rows per pixel
HR = 9               # rows per half-chunk
TEX = 24             # texels per window row
EL = TEX * 96        # 2304 bf16 elems per descriptor (4608 B)
NIH = HR * 128       # 1152 gather positions per half-chunk
NIH16 = NIH // 16    # 72
GSTRIDE = 7690       # groups per batch image (7680 + pad)
NGROUP = 4 * GSTRIDE + 16
POS = HR * TEX       # 216 dot outputs per pixel per half

f32 = mybir.dt.float32
bf16 = mybir.dt.bfloat16
i16 = mybir.dt.int16

_CACHE = {}


# ---------------------------------------------------------------- geometry
def _part1_jax(R, T, initial_flow):
    import jax
    import jax.numpy as jnp

    cpu = jax.devices("cpu")[0]

    def f(R, T, initial_flow):
        K = np.zeros((3, 3), np.float64)
        K[0, 0] = 0.89115971 * W
        K[0, 2] = 0.5 * W
        K[1, 1] = 1.18821287 * H
        K[1, 2] = 0.5 * H
        K[2, 2] = 1.0
        Kn = K.astype(np.float32)
        Ki = np.linalg.inv(K).astype(np.float32)
        jj, ii = np.meshgrid(np.arange(W), np.arange(H))
        pix_h = np.stack([jj, ii, np.ones_like(jj)], -1).reshape(-1, 3).astype(np.float32)
        pixel_dir = jnp.asarray(pix_h @ Ki.T)
        pixel_loc = jnp.asarray(np.stack([jj, ii], -1).astype(np.float32))
        Kj = jnp.asarray(Kn)
        KR = jnp.einsum('ij,bjk->bik', Kj, R)
        first_part = jnp.einsum('bij,nj->bni', KR, pixel_dir)
        second_part = jnp.einsum('ij,bjk->bik', Kj, T)[:, :, 0][:, None, :]

        def safe(d):
            return jnp.where(jnp.abs(d) < 1e-6, 1e-6, d)

        end_point = first_part[..., :2] / safe(first_part[..., 2:3])
        space_point = first_part * 10.0 + second_part
        project_point = space_point[..., :2] / safe(space_point[..., 2:3])
        diff = project_point - end_point
        para = diff / jnp.maximum(jnp.linalg.norm(diff, axis=-1, keepdims=True), 1e-12)
        perp = jnp.stack([-para[..., 1], para[..., 0]], axis=-1)
        para_r = para.reshape(B, H, W, 2)
        perp_r = perp.reshape(B, H, W, 2)
        end_r = end_point.reshape(B, H, W, 2)
        flow_point = pixel_loc[None] + jnp.transpose(initial_flow, (0, 2, 3, 1))
        nearest_k = jnp.sum((flow_point - end_r) * para_r, axis=3, keepdims=True)
        initial_loc = end_r + nearest_k * para_r
        epipolar_flow = jnp.transpose(initial_loc - pixel_loc[None], (0, 3, 1, 2))
        para_out = jnp.transpose(para_r, (0, 3, 1, 2))
        return initial_loc, para_r, perp_r, epipolar_flow, para_out

    with jax.default_device(cpu):
        args = [jax.device_put(np.asarray(x), cpu) for x in (R, T, initial_flow)]
        out = jax.jit(f, backend="cpu")(*args)
    return [np.asarray(x) for x in out]


def geometry(R, T, initial_flow):
    """Windows, gather groups, tap positions and weights (all fp32-exact
    vs the reference)."""
    initial_loc, para, perp, eflow, para_out = _part1_jax(R, T, initial_flow)
    initial_loc = initial_loc.reshape(B, HW, 2)
    para = para.reshape(B, HW, 2)
    perp = perp.reshape(B, HW, 2)
    offsets = np.array([[p, q] for p in MAXD for q in MIND], np.float32)  # (O,2)

    base = initial_loc + perp                                    # (B,HW,2)
    one, two, half = np.float32(1.0), np.float32(2.0), np.float32(0.5)
    Wf, Hf = np.float32(W), np.float32(H)
    # g = base + para_i*para + perp_i ; normalize/denormalize as reference
    pa_i = offsets[:, 0][None, :, None]                          # (1,O,1)
    pe_i = offsets[:, 1][None, :, None]
    gx = base[:, None, :, 0] + pa_i * para[:, None, :, 0] + pe_i  # (B,O,HW)
    gy = base[:, None, :, 1] + pa_i * para[:, None, :, 1] + pe_i
    gxn = two * gx / (Wf - one) - one
    gyn = two * gy / (Hf - one) - one
    gx = ((gxn + one) * Wf - one) * half
    gy = ((gyn + one) * Hf - one) * half
    gx = np.nan_to_num(gx, nan=1e9, posinf=1e9, neginf=-1e9)
    gy = np.nan_to_num(gy, nan=1e9, posinf=1e9, neginf=-1e9)
    x0 = np.floor(gx)
    y0 = np.floor(gy)
    wx = gx - x0
    wy = gy - y0

    in_x = (x0 >= 0) & (x0 <= W - 2)
    left = x0 == -1
    right = x0 == np.float32(W - 1)
    xlive = (x0 >= -1) & (x0 <= W - 1)
    ws0 = np.where(in_x, one - wx, np.where(left, wx, 0.0)).astype(np.float32)
    ws1 = np.where(in_x, wx, np.where(right, one - wx, 0.0)).astype(np.float32)
    xb = np.clip(x0, 0, W - 2).astype(np.int32)                  # (B,O,HW)

    ylive = [(y0 + r >= 0) & (y0 + r <= H - 1) for r in (0, 1)]
    yc = [np.clip(y0 + r, 0, H - 1).astype(np.int32) for r in (0, 1)]
    wrow = [np.where(ylive[r], (one - wy) if r == 0 else wy, 0.0).astype(np.float32)
            for r in (0, 1)]

    slive = [xlive & ylive[r] for r in (0, 1)]                   # (B,O,HW)
    anylive = slive[0] | slive[1]
    pixlive = anylive.any(axis=1)                                # (B,HW)

    big = np.int32(1 << 20)
    xb_m = np.where(anylive, xb, big)
    xmin = xb_m.min(axis=1)                                      # (B,HW)
    ys = [np.where(slive[r], yc[r], big) for r in (0, 1)]
    ymin = np.minimum(ys[0].min(axis=1), ys[1].min(axis=1))      # (B,HW)
    WX = np.clip((xmin // 4) * 4, 0, W - TEX).astype(np.int32)
    WY = np.clip(ymin, 0, H - ROWS).astype(np.int32)

    # taps: (B,O,2r,2s,HW) weight + position-in-window
    wt = np.empty((B, O, 2, 2, HW), np.float32)
    pos = np.zeros((B, O, 2, 2, HW), np.int16)
    for r in (0, 1):
        dy = yc[r] - WY[:, None]                                 # (B,O,HW)
        dx = xb - WX[:, None]
        p_base = dy * TEX + dx
        for s in (0, 1):
            w = wrow[r] * (ws0 if s == 0 else ws1)
            w = np.where(slive[r], w, 0.0)
            wt[:, :, r, s, :] = w
            pos[:, :, r, s, :] = np.where(w != 0, p_base + s, 0).astype(np.int16)
    wt /= np.float32(C)
    return eflow, para_out, pixlive, WX, WY, pos, wt


# ---------------------------------------------------------------- device
def build_program(nhalf):
    nc = bacc.Bacc("TRN2", debug=False)
    src_d = nc.dram_tensor("src", [NGROUP, 384], bf16, kind="ExternalInput")
    idx_d = nc.dram_tensor("idxs", [nhalf, 128, NIH16], i16, kind="ExternalInput")
    lv_d = nc.dram_tensor("lv", [nhalf, 128, 96], bf16, kind="ExternalInput")
    d_out = nc.dram_tensor("dvals", [nhalf, 128, POS], f32, kind="ExternalOutput")

    src_ap = bass.AP(src_d[:].tensor, 0, [[384, NGROUP - 12], [1, EL]])
    fence_ap = bass.AP(src_d[:].tensor, 0, [[384, NGROUP - 12], [1, 128]])

    G = [nc.alloc_sbuf_tensor(f"g{i}", [128, HR, EL], bf16) for i in range(4)]
    idx_s = [nc.alloc_sbuf_tensor(f"ix{i}", [128, NIH16], i16) for i in range(4)]
    lv_s = [nc.alloc_sbuf_tensor(f"lv{i}", [128, 96], bf16) for i in range(4)]
    d_s = [nc.alloc_sbuf_tensor(f"d{i}", [128, POS], f32) for i in range(4)]
    fence_s = nc.alloc_sbuf_tensor("fen", [128, HR, 128], bf16)

    s_idx = nc.alloc_semaphore("s_idx")
    s_lv = nc.alloc_semaphore("s_lv")
    s_g = nc.alloc_semaphore("s_g")
    s_v = nc.alloc_semaphore("s_v")
    s_out = nc.alloc_semaphore("s_out")
    s_mB = nc.alloc_semaphore("s_mB")
    PF = HR * EL

    with nc.Block() as blk:

        def g_aps(k, off, width):
            gt = G[k % 4][:].tensor
            return bass.AP(gt, off, [[PF, 128], [96, POS], [1, width]])

        def l_aps(k, off, width):
            lt = lv_s[k % 4][:].tensor
            return bass.AP(lt, off, [[96, 128], [0, POS], [1, width]])

        @blk.vector
        def _(v):
            for k in range(nhalf):
                v.wait_ge(s_g, 16 * (k + 3))   # gather k+2 issued => k retired
                v.wait_ge(s_lv, 16 * (k + 1))
                if k >= 4:
                    v.wait_ge(s_out, 16 * (k - 3))
                gF = g_aps(k, 0, 96)
                v.tensor_mul(gF, gF, l_aps(k, 0, 96))
                gA = g_aps(k, 0, 48)
                v.tensor_add(gA, gA, g_aps(k, 48, 48))   # pairs (c, c+48)
                gA2 = g_aps(k, 0, 24)
                v.tensor_add(gA2, gA2, g_aps(k, 24, 24))  # quads
                v.tensor_reduce(
                    d_s[k % 4][:], gA2,
                    mybir.AxisListType.X, mybir.AluOpType.add,
                ).then_inc(s_v, 1)

        @blk.gpsimd
        def _(g):
            g.load_library(mlp)
            for k in range(nhalf):
                g.wait_ge(s_idx, 16 * (k + 1))
                if k >= 4:
                    g.wait_ge(s_v, k - 3)      # vector k-4 done with G[k%4]
                g.dma_gather(
                    G[k % 4][:], src_ap, idx_s[k % 4][:],
                    NIH, NIH, EL, elem_step=384, transpose=False,
                    single_packet=False,
                ).then_inc(s_g, 16)
            for _ in range(2):                  # tail fences
                g.dma_gather(
                    fence_s[:], fence_ap, idx_s[(nhalf - 1) % 4][:],
                    NIH, NIH, 128, elem_step=384, transpose=False,
                    single_packet=False,
                ).then_inc(s_g, 16)

        @blk.sync
        def _(sy):
            for k in range(min(4, nhalf)):
                sy.dma_start(idx_s[k][:], idx_d[k]).then_inc(s_idx, 16)
                sy.dma_start(lv_s[k][:], lv_d[k]).then_inc(s_lv, 16)
            for k in range(nhalf):
                sy.wait_ge(s_v, k + 1)
                if k + 4 < nhalf:
                    sy.dma_start(idx_s[k % 4][:], idx_d[k + 4]).then_inc(s_idx, 16)
                    sy.dma_start(lv_s[k % 4][:], lv_d[k + 4]).then_inc(s_lv, 16)
                sy.dma_start(d_out[k], d_s[k % 4][:]).then_inc(s_out, 16)
            sy.wait_ge(s_out, 16 * nhalf)

    nc.compile()
    nc.finalize()
    return nc


# ---------------------------------------------------------------- host glue
def prepare(imgL, imgR, R, T, initial_flow):
    eflow, para_out, pixlive, WX, WY, pos, wt = geometry(R, T, initial_flow)

    # gather source: all 4 images, tight (HW, 96) bf16, 768B groups
    src = np.zeros((NGROUP, 384), ml_dtypes.bfloat16)
    for b in range(B):
        t = np.moveaxis(imgR[b], 0, -1).reshape(HW, 96).astype(ml_dtypes.bfloat16)
        src[b * GSTRIDE: b * GSTRIDE + HW // 4] = t.reshape(HW // 4, 384)

    # global live-pixel compaction
    blist, plist = np.nonzero(pixlive)                  # (nlive,) each
    nlive = blist.size
    ncH = max(1, -(-nlive // (8 * 128)))                # full chunks per core
    cap = 8 * ncH * 128
    bl = np.zeros(cap, np.int64)
    pl = np.zeros(cap, np.int64)
    valid = np.zeros(cap, bool)
    bl[:nlive] = blist
    pl[:nlive] = plist
    valid[:nlive] = True

    # entry -> 18 window-row group indices
    wx_e = WX[bl, pl].astype(np.int64)
    wy_e = WY[bl, pl].astype(np.int64)
    g0 = bl * GSTRIDE + (wy_e * W + wx_e) // 4          # (cap,)
    rows = g0[:, None] + np.arange(ROWS)[None, :] * (W // 4)   # (cap,18)
    rows = np.where(valid[:, None], rows, 0)
    assert rows.max() < 32767

    # imgL vectors per entry
    py = pl // W
    px = pl % W
    lv_all = imgL[bl, :, py, px].astype(ml_dtypes.bfloat16)     # (cap,96)

    nhalf = 2 * ncH
    in_maps = []
    percore = ncH * 128
    for c in range(8):
        sl = slice(c * percore, (c + 1) * percore)
        r_c = rows[sl].reshape(ncH, 128, 2, HR)          # (nc,px,half,9)
        lists = r_c.transpose(0, 2, 3, 1).reshape(nhalf, NIH).astype(np.int16)
        idx_w = lists.reshape(nhalf, NIH16, 16).transpose(0, 2, 1)
        idx_full = np.ascontiguousarray(np.tile(idx_w, (1, 8, 1)))
        lv_c = lv_all[sl].reshape(ncH, 128, 96)
        lv_h = np.ascontiguousarray(
            np.repeat(lv_c[:, None], 2, axis=1).reshape(nhalf, 128, 96))
        in_maps.append({"src": src, "idxs": idx_full, "lv": lv_h})

    ctx = dict(eflow=eflow, para_out=para_out, pos=pos, wt=wt,
               bl=bl, pl=pl, valid=valid, nlive=nlive, ncH=ncH,
               pixlive=pixlive)
    return in_maps, ctx


def combine(results, ctx):
    ncH = ctx["ncH"]
    cap = 8 * ncH * 128
    dfull = np.empty((cap, 2 * POS), np.float32)
    percore = ncH * 128
    for c in range(8):
        dv = results[c]["dvals"].reshape(ncH, 2, 128, POS)
        dfull[c * percore:(c + 1) * percore] = (
            dv.transpose(0, 2, 1, 3).reshape(percore, 2 * POS))

    # entry map (B,HW) -> compacted row (0 for dead; weight 0 there)
    entmap = np.zeros((B, HW), np.int64)
    bl, pl, valid = ctx["bl"], ctx["pl"], ctx["valid"]
    entmap[bl[valid], pl[valid]] = np.nonzero(valid)[0]

    pos = ctx["pos"]                                    # (B,O,2,2,HW) i16
    wt = ctx["wt"]                                      # (B,O,2,2,HW) f32
    pos_r = pos.transpose(0, 4, 1, 2, 3).reshape(B, HW, O * 4)
    wt_r = wt.transpose(0, 4, 1, 2, 3).reshape(B, HW, O * 4)
    vals = dfull[entmap[:, :, None], pos_r.astype(np.int64)]    # (B,HW,O*4)
    corr = (vals * wt_r).reshape(B, HW, O, 4).sum(axis=3)       # (B,HW,O)
    corr = corr.transpose(0, 2, 1)                              # (B,O,HW)

    out = np.empty((B, 4 + O, H, W), np.float32)
    out[:, 0:2] = ctx["eflow"]
    out[:, 2:4] = ctx["para_out"]
    out[:, 4:] = corr.reshape(B, O, H, W)
    return out


def kernel(imgL, imgR, R, T, initial_flow):
    imgL = np.asarray(imgL)
    imgR = np.asarray(imgR)
    R = np.asarray(R)
    T = np.asarray(T)
    initial_flow = np.asarray(initial_flow)

    in_maps, ctx = prepare(imgL, imgR, R, T, initial_flow)
    nhalf = 2 * ctx["ncH"]
    key = ("nc", nhalf)
    if key not in _CACHE:
        _CACHE[key] = build_program(nhalf)
    nc = _CACHE[key]
    _CACHE["last"] = (nc, in_maps)

    res = bass_utils.run_bass_kernel_spmd(nc, in_maps, core_ids=list(range(8)),
                                          trace=False)
    return combine(res.results, ctx)
